# revision 6
# baseline (speedup 1.0000x reference)
"""Trainium2 Bass kernel for channel attention (1x1 conv -> depthwise 3x3 ->
per-head channel attention over pixels -> 1x1 projection).

Data-parallel over batch: 8 images -> 8 NeuronCores, no collectives.
Self-contained: hardcodes shapes from the problem spec.
"""
import sys

sys.path.insert(0, "/opt/trn_rl_repo")

import numpy as np  # noqa: E402

import concourse.bacc as bacc  # noqa: E402
import concourse.mybir as mybir  # noqa: E402
from concourse import masks  # noqa: E402
from concourse.tile import TileContext  # noqa: E402
from concourse.bass_utils import run_bass_kernel_spmd  # noqa: E402

dt = mybir.dt
Alu = mybir.AluOpType
Act = mybir.ActivationFunctionType
Axis = mybir.AxisListType
F32R = dt.float32r

# geometry
P = 128
W = 128            # image row length
HW = 16384         # pixels per image
MEGA = 2048        # pixels per mega-tile (16 image rows)
ROWS = MEGA // W   # 16
NMEGA = HW // MEGA  # 8
ZW = MEGA + 2 * W  # 2304 data cols: mega + 1 halo row each side
ZT = 2432          # z tile width: 1 left pad + ZW data + right pad (19 rows)
ZB = 129           # z col of first output pixel (out c -> z col c + ZB)
CI = 384
HEADS = 8
CH = 48            # channels per head
NCORES = 8

# dwconv tap split: (dy, dx); flat shift = 128*dy + dx on the padded z layout
PE_TAPS = [(0, 0), (0, -1), (0, 1), (-1, 0), (-1, -1)]
DVE_TAPS = [(1, 0), (1, -1), (1, 1), (-1, 1)]  # first must have dx == 0
ALL_TAPS = PE_TAPS + DVE_TAPS

# banded gram layout: for q-chunk g (128 q channels), k-channel band
BANDS = [(0, 144), (96, 192), (240, 144)]  # (start, width) in k channels
# per-head partition segments: (gchunk, p0, p1, col0, head)
SEGMENTS = [
    (0, 0, 48, 0, 0), (0, 48, 96, 48, 1), (0, 96, 128, 96, 2),
    (1, 0, 16, 0, 2), (1, 16, 64, 48, 3), (1, 64, 112, 96, 4),
    (1, 112, 128, 144, 5),
    (2, 0, 32, 0, 5), (2, 32, 80, 48, 6), (2, 80, 128, 96, 7),
]


def _mega_geometry(m):
    """(w0, conv_chunks) for mega m. w0: DRAM px of window col 0 (may be <0).
    conv_chunks: (off, n) over valid window cols [z_lo, z_hi)."""
    w0 = MEGA * m - W
    z_lo = W if m == 0 else 0
    z_hi = ZW - W if m == NMEGA - 1 else ZW
    chunks = []
    off = z_lo
    while off < z_hi:
        n = min(512, z_hi - off)
        chunks.append((off, n))
        off += n
    return w0, chunks


def build_nc():
    nc = bacc.Bacc("TRN2", target_bir_lowering=False, debug=False)

    xin = nc.dram_tensor("x", [CI, HW], F32R, kind="ExternalInput").ap()
    wqkv = nc.dram_tensor("wqkv", [3 * CI, CI], dt.float32, kind="ExternalInput").ap()
    bqkv = nc.dram_tensor("bqkv", [3 * CI], dt.float32, kind="ExternalInput").ap()
    wdw = nc.dram_tensor("wdw", [3 * CI, 9], dt.float32, kind="ExternalInput").ap()
    bdw = nc.dram_tensor("bdw", [3 * CI], dt.float32, kind="ExternalInput").ap()
    wp = nc.dram_tensor("wp", [CI, CI], dt.float32, kind="ExternalInput").ap()
    bp = nc.dram_tensor("bp", [CI], dt.float32, kind="ExternalInput").ap()
    temp = nc.dram_tensor("temp", [HEADS], dt.float32, kind="ExternalInput").ap()
    yout = nc.dram_tensor("y", [CI, HW], dt.float32, kind="ExternalOutput").ap()

    with TileContext(nc) as tc:
        _build(tc, nc, xin, wqkv, bqkv, wdw, bdw, wp, bp, temp, yout)
    nc.compile()
    return nc


def _conv1x1(nc, ps_pool, wqkvT, xw, z, g, bias, chunks):
    """1x1 conv for output-channel chunk g into padded z tile."""
    for off, n in chunks:
        psc = ps_pool.tile([P, 512], dt.float32, tag="psc", name="psc")
        for c3 in range(3):
            nc.tensor.matmul(
                psc[:, :n],
                wqkvT[:, c3, g * P:(g + 1) * P],
                xw[:, c3, off:off + n],
                start=(c3 == 0), stop=(c3 == 2))
        nc.scalar.activation(z[:, off + 1:off + 1 + n], psc[:, :n],
                             Act.Identity, bias=bias)


def _dwconv(nc, ps_dw, acc_pool, z, g, dww, dww_neg, diags, ident_r,
            bias, evac_out, evac_dtype_tile):
    """Depthwise 3x3 on padded z -> 4 psum chunks, evacuated via
    evac_out(c, psum_ap). Taps split across PE (diag matmuls) and DVE."""
    zf = z.bitcast(dt.float32)
    z2 = zf.rearrange("p (r x) -> p r x", x=W)  # 19 rows
    acc = acc_pool.tile([P, MEGA], F32R, tag="acc", name="acc")
    accf = acc.bitcast(dt.float32)
    for i, (dy, dx) in enumerate(DVE_TAPS):
        t = (dy + 1) * 3 + (dx + 1)
        wsc = dww[:, g, t:t + 1]
        src = zf[:, ZB + 128 * dy + dx: ZB + 128 * dy + dx + MEGA]
        if i == 0:
            assert dx == 0
            nc.vector.tensor_scalar_mul(acc[:], src, wsc)
        else:
            nc.vector.scalar_tensor_tensor(acc[:], src, wsc, accf[:],
                                           Alu.mult, Alu.add)
    # wrap corrections for every dx != 0 tap (PE taps included: acc is
    # merged into the psum, so linear corrections can all land on acc)
    acc3 = acc.rearrange("p (r x) -> p r x", x=W)
    acc3f = accf.rearrange("p (r x) -> p r x", x=W)
    for (dy, dx) in ALL_TAPS:
        if dx == 0:
            continue
        t = (dy + 1) * 3 + (dx + 1)
        wneg = dww_neg[:, g, t:t + 1]
        if dx == -1:
            o_ap, of_ap = acc3[:, :, 0:1], acc3f[:, :, 0:1]
            s_ap = z2[:, dy + 1:dy + 17, 0:1]
        else:
            o_ap, of_ap = acc3[:, :, W - 1:W], acc3f[:, :, W - 1:W]
            s_ap = z2[:, dy + 2:dy + 18, 1:2]
        nc.vector.scalar_tensor_tensor(o_ap, s_ap, wneg, of_ap,
                                       Alu.mult, Alu.add)
    for c in range(4):
        psd = ps_dw.tile([P, 512], dt.float32, tag="psd", name="psd")
        for i, (dy, dx) in enumerate(PE_TAPS):
            s0 = ZB + 512 * c + 128 * dy + dx
            nc.tensor.matmul(psd[:], diags[:, g, i, :],
                             z[:, s0:s0 + 512],
                             start=(i == 0), stop=False)
        nc.tensor.matmul(psd[:], ident_r[:],
                         acc[:, 512 * c:512 * (c + 1)],
                         start=False, stop=True)
        nc.scalar.activation(evac_out(c), psd[:], Act.Identity, bias=bias)


def _build(tc, nc, xin, wqkv, bqkv, wdw, bdw, wp, bp, temp, yout):
    from contextlib import ExitStack

    es = ExitStack()
    with es:
        const = es.enter_context(tc.tile_pool(name="const", bufs=1))
        wsetup = es.enter_context(tc.tile_pool(name="wsetup", bufs=2))
        xw_pool = es.enter_context(tc.tile_pool(name="xw", bufs=2))
        z_pool = es.enter_context(tc.tile_pool(name="z", bufs=2))
        acc_pool = es.enter_context(tc.tile_pool(name="acc", bufs=2))
        small = es.enter_context(tc.tile_pool(name="small", bufs=2))
        ps_conv = es.enter_context(tc.tile_pool(name="ps_conv", bufs=2, space="PSUM"))
        ps_dw = es.enter_context(tc.tile_pool(name="ps_dw", bufs=2, space="PSUM"))

        # ---------------- constants / weights ----------------
        ident_f = const.tile([P, P], dt.float32)
        masks.make_identity(nc, ident_f[:])
        ident_bf = const.tile([P, P], dt.bfloat16)
        masks.make_identity(nc, ident_bf[:])
        ident_r = const.tile([P, P], F32R)
        nc.scalar.activation(ident_r[:], ident_f[:], Act.Copy)

        wqkvT = const.tile([P, 3, 3 * CI], F32R)   # [ci_in, ci_chunk, co]
        for o9 in range(9):
            wtmp = wsetup.tile([P, CI], dt.float32, tag="wtmp", name="wtmp")
            nc.sync.dma_start(wtmp[:], wqkv[o9 * P:(o9 + 1) * P, :])
            for c3 in range(3):
                pst = ps_conv.tile([P, 512], dt.float32, tag="psc", name="pst")
                nc.tensor.transpose(pst[:, :P], wtmp[:, c3 * P:(c3 + 1) * P],
                                    ident_f[:])
                nc.scalar.activation(wqkvT[:, c3, o9 * P:(o9 + 1) * P],
                                     pst[:, :P], Act.Copy)

        wpT = const.tile([P, 3, CI], F32R)  # [e, e_chunk, c]
        for mc in range(3):
            wtmp = wsetup.tile([P, CI], dt.float32, tag="wtmp", name="wtmp")
            nc.sync.dma_start(wtmp[:], wp[mc * P:(mc + 1) * P, :])
            for ec in range(3):
                pst = ps_conv.tile([P, 512], dt.float32, tag="psc", name="pst")
                nc.tensor.transpose(pst[:, :P], wtmp[:, ec * P:(ec + 1) * P],
                                    ident_f[:])
                nc.scalar.activation(wpT[:, ec, mc * P:(mc + 1) * P],
                                     pst[:, :P], Act.Copy)

        dww = const.tile([P, 9, 9], dt.float32)  # [ch, g, tap]
        for g in range(9):
            nc.sync.dma_start(dww[:, g, :], wdw[g * P:(g + 1) * P, :])
        dww_neg = const.tile([P, 9, 9], dt.float32)
        nc.vector.tensor_scalar_mul(dww_neg[:], dww[:], -1.0)

        bqkv_sb = const.tile([P, 9], dt.float32)
        nc.sync.dma_start(bqkv_sb[:], bqkv.rearrange("(g p) -> p g", p=P))
        bdw_sb = const.tile([P, 9], dt.float32)
        nc.sync.dma_start(bdw_sb[:], bdw.rearrange("(g p) -> p g", p=P))
        bp_sb = const.tile([P, 3], dt.float32)
        nc.sync.dma_start(bp_sb[:], bp.rearrange("(g p) -> p g", p=P))

        # per-q-channel temperature via broadcast segment DMAs
        tmap = const.tile([P, 3], dt.float32)
        for (gq, p0, p1, c0, h) in SEGMENTS:
            src = temp[h:h + 1].rearrange("(a b) -> a b", b=1)
            nc.sync.dma_start(tmap[p0:p1, gq:gq + 1],
                              src.to_broadcast([p1 - p0, 1]))

        # diag tap-weight matrices for PE dwconv taps
        diags = const.tile([P, 9, len(PE_TAPS), P], F32R)
        for g in range(9):
            for i, (dy, dx) in enumerate(PE_TAPS):
                t = (dy + 1) * 3 + (dx + 1)
                nc.vector.tensor_scalar_mul(diags[:, g, i, :], ident_f[:],
                                            dww[:, g, t:t + 1])

        ones_col = const.tile([P, 1], dt.float32)
        nc.gpsimd.memset(ones_col[:], 1.0)
        ones_row = const.tile([1, P], dt.float32)
        nc.gpsimd.memset(ones_row[:], 1.0)

        sqp = const.tile([P, 6, NMEGA], dt.float32)
        a_bd = const.tile([P, 3, CI], F32R)
        mt = const.tile([P, 3, CI], F32R)
        logit_all = const.tile([P, 3, 192], dt.float32)

        # ---------------- phase 1: q/k ----------------
        with ExitStack() as qk_es:
            gram_pool = qk_es.enter_context(
                tc.tile_pool(name="gram", bufs=1, space="PSUM"))
            ps_t = qk_es.enter_context(
                tc.tile_pool(name="ps_t", bufs=1, space="PSUM"))
            qbf_pool = qk_es.enter_context(tc.tile_pool(name="qbf", bufs=3))
            qkt_pool = qk_es.enter_context(tc.tile_pool(name="qkt", bufs=1))
            scr_pool = qk_es.enter_context(tc.tile_pool(name="scr", bufs=2))

            grams = [gram_pool.tile([P, BANDS[g][1]], dt.float32, tag=f"g{g}",
                                    name=f"gram{g}")
                     for g in range(3)]

            for m in range(NMEGA):
                w0, chunks = _mega_geometry(m)
                xw = xw_pool.tile([P, 3, ZW], F32R, tag="xw", name="xw")
                xs = max(w0, 0)
                xe = min(w0 + ZW, HW)
                for c3 in range(3):
                    nc.sync.dma_start(xw[:, c3, xs - w0:xe - w0],
                                      xin[c3 * P:(c3 + 1) * P, xs:xe])

                qkt = qkt_pool.tile([P, ROWS, 768], dt.bfloat16, tag="qkt",
                                    name="qkt")

                for g in range(6):
                    z = z_pool.tile([P, ZT], F32R, tag="z", name="z")
                    nc.vector.tensor_scalar_mul(z[:, 0:1], ident_f[:, 0:1], 0.0)
                    nc.vector.tensor_scalar_mul(z[:, 1 + ZW:ZT],
                                                ident_f[:, 0:ZT - ZW - 1], 0.0)
                    if m == 0:
                        nc.vector.tensor_scalar_mul(z[:, 1:1 + W], ident_f[:],
                                                    0.0)
                    if m == NMEGA - 1:
                        nc.vector.tensor_scalar_mul(z[:, 1 + ZW - W:1 + ZW],
                                                    ident_f[:], 0.0)
                    _conv1x1(nc, ps_conv, wqkvT, xw, z, g,
                             bqkv_sb[:, g:g + 1], chunks)

                    qb = qbf_pool.tile([P, MEGA], dt.bfloat16, tag="qb",
                                       name="qb")
                    _dwconv(nc, ps_dw, acc_pool, z, g, dww, dww_neg, diags,
                            ident_r, bdw_sb[:, g:g + 1],
                            lambda c: qb[:, 512 * c:512 * (c + 1)], None)

                    # sum of squares for the l2 norms
                    scr = scr_pool.tile([P, MEGA], dt.bfloat16, tag="scr",
                                        name="scr")
                    nc.scalar.activation(scr[:], qb[:], Act.Square,
                                         accum_out=sqp[:, g, m:m + 1])

                    # transpose to [px, ch]
                    for c in range(4):
                        pstr = ps_t.tile([P, 512], dt.bfloat16, tag="pstr",
                                         name="pstr")
                        for j in range(4):
                            blk = 4 * c + j
                            nc.tensor.transpose(pstr[:, j * P:(j + 1) * P],
                                                qb[:, blk * P:(blk + 1) * P],
                                                ident_bf[:])
                        nc.scalar.activation(
                            qkt[:, 4 * c:4 * c + 4, g * P:(g + 1) * P],
                            pstr[:], Act.Copy)

                # banded gram accumulation over px chunks
                for j in range(ROWS):
                    for gq in range(3):
                        bs, bw = BANDS[gq]
                        nc.tensor.matmul(
                            grams[gq][:],
                            qkt[:, j, gq * P:(gq + 1) * P],
                            qkt[:, j, CI + bs:CI + bs + bw],
                            start=(m == 0 and j == 0),
                            stop=(m == NMEGA - 1 and j == ROWS - 1))

            # ------------- phase 2a: norms, logits, softmax, A_bd -------------
            sq = const.tile([P, 6], dt.float32)
            nc.vector.tensor_reduce(sq[:], sqp[:], Axis.X, Alu.add)
            nrm = const.tile([P, 6], dt.float32)
            nc.scalar.activation(nrm[:], sq[:], Act.Sqrt)
            inv = const.tile([P, 6], dt.float32)
            nc.vector.reciprocal(inv[:], nrm[:])
            rowscale = const.tile([P, 3], dt.float32)
            nc.vector.tensor_tensor(rowscale[:], tmap[:], inv[:, 0:3], Alu.mult)

            # broadcast 1/||k|| along partitions: invk_bc[p, d] = inv_nk[d]
            invk_row = const.tile([1, CI], dt.float32)
            for g in range(3):
                dtmp = small.tile([P, P], dt.float32, tag="dtmp", name="dtmp")
                nc.vector.tensor_scalar_mul(dtmp[:], ident_f[:],
                                            inv[:, 3 + g:4 + g])
                pst = ps_conv.tile([P, 512], dt.float32, tag="psc", name="pst")
                nc.tensor.matmul(pst[:1, :P], ones_col[:], dtmp[:],
                                 start=True, stop=True)
                nc.scalar.activation(invk_row[:, g * P:(g + 1) * P],
                                     pst[:1, :P], Act.Copy)
            invk_bc = const.tile([P, CI], dt.float32)
            pst2 = ps_dw.tile([P, 512], dt.float32, tag="psd", name="pst2")
            nc.tensor.matmul(pst2[:, :CI], ones_row[:], invk_row[:],
                             start=True, stop=True)
            nc.scalar.activation(invk_bc[:], pst2[:, :CI], Act.Copy)

            for gq in range(3):
                bs, bw = BANDS[gq]
                nc.scalar.activation(logit_all[:, gq, :bw], grams[gq][:],
                                     Act.Identity,
                                     scale=rowscale[:, gq:gq + 1])
                nc.vector.tensor_tensor(logit_all[:, gq, :bw],
                                        logit_all[:, gq, :bw],
                                        invk_bc[:, bs:bs + bw], Alu.mult)

            # head-aligned softmax: gather [48 qc, 8 heads, 48 kc]
            L = const.tile([CH, HEADS, CH], dt.float32)
            for (gq, p0, p1, c0, h) in SEGMENTS:
                lp0 = 128 * gq + p0 - CH * h
                nc.sync.dma_start(L[lp0:lp0 + (p1 - p0), h, :],
                                  logit_all[p0:p1, gq, c0:c0 + CH])
            nmx = const.tile([CH, HEADS], dt.float32)
            nc.vector.tensor_reduce(nmx[:], L[:], Axis.X, Alu.max, negate=True)
            nc.vector.tensor_tensor(L[:], L[:],
                                    nmx[:, :, None].to_broadcast(
                                        (CH, HEADS, CH)), Alu.add)
            Lx = const.tile([CH, HEADS, CH], dt.float32)
            nc.scalar.activation(Lx[:], L[:], Act.Exp)
            ssum = const.tile([CH, HEADS], dt.float32)
            nc.vector.tensor_reduce(ssum[:], Lx[:], Axis.X, Alu.add)
            rinv = const.tile([CH, HEADS], dt.float32)
            nc.vector.reciprocal(rinv[:], ssum[:])
            Lxr = const.tile([CH, HEADS, CH], F32R)
            nc.vector.tensor_tensor(Lxr[:], Lx[:],
                                    rinv[:, :, None].to_broadcast(
                                        (CH, HEADS, CH)), Alu.mult)

            for gz in range(3):
                nc.vector.tensor_scalar_mul(
                    a_bd[:, gz, :], wpT[:, 0, :].bitcast(dt.float32), 0.0)
            for (gq, p0, p1, c0, h) in SEGMENTS:
                lp0 = 128 * gq + p0 - CH * h
                nc.sync.dma_start(a_bd[p0:p1, gq, CH * h:CH * (h + 1)],
                                  Lxr[lp0:lp0 + (p1 - p0), h, :])

        # ---------------- phase 2b: MT = (Wp @ A_bd)^T ----------------
        for dc in range(3):
            psm = ps_conv.tile([P, 512], dt.float32, tag="psc", name="psm")
            for ec in range(3):
                nc.tensor.matmul(psm[:, :CI],
                                 a_bd[:, ec, dc * P:(dc + 1) * P],
                                 wpT[:, ec, :],
                                 start=(ec == 0), stop=(ec == 2))
            nc.scalar.activation(mt[:, dc, :], psm[:, :CI], Act.Copy)

        # ---------------- phase 3: v and output ----------------
        with ExitStack() as v_es:
            vsb_pool = v_es.enter_context(tc.tile_pool(name="vsb", bufs=2))
            ych_pool = v_es.enter_context(tc.tile_pool(name="ych", bufs=2))
            ps_mv = v_es.enter_context(
                tc.tile_pool(name="ps_mv", bufs=2, space="PSUM"))

            for m in range(NMEGA):
                w0, chunks = _mega_geometry(m)
                xw = xw_pool.tile([P, 3, ZW], F32R, tag="xw", name="xw")
                xs = max(w0, 0)
                xe = min(w0 + ZW, HW)
                for c3 in range(3):
                    nc.sync.dma_start(xw[:, c3, xs - w0:xe - w0],
                                      xin[c3 * P:(c3 + 1) * P, xs:xe])

                vsb = vsb_pool.tile([P, 3, MEGA], F32R, tag="vsb",
                                    name="vsb")

                for gv in range(3):
                    g = 6 + gv
                    z = z_pool.tile([P, ZT], F32R, tag="z", name="z")
                    nc.vector.tensor_scalar_mul(z[:, 0:1], ident_f[:, 0:1], 0.0)
                    nc.vector.tensor_scalar_mul(z[:, 1 + ZW:ZT],
                                                ident_f[:, 0:ZT - ZW - 1], 0.0)
                    if m == 0:
                        nc.vector.tensor_scalar_mul(z[:, 1:1 + W], ident_f[:],
                                                    0.0)
                    if m == NMEGA - 1:
                        nc.vector.tensor_scalar_mul(z[:, 1 + ZW - W:1 + ZW],
                                                    ident_f[:], 0.0)
                    _conv1x1(nc, ps_conv, wqkvT, xw, z, g,
                             bqkv_sb[:, g:g + 1], chunks)

                    _dwconv(nc, ps_dw, acc_pool, z, g, dww, dww_neg, diags,
                            ident_r, bdw_sb[:, g:g + 1],
                            lambda c, gv=gv: vsb[:, gv, 512 * c:512 * (c + 1)],
                            None)

                # y = M @ v + bias
                for mc in range(3):
                    for c in range(4):
                        psy = ps_mv.tile([P, 512], dt.float32, tag="psy",
                                         name="psy")
                        for kd in range(3):
                            nc.tensor.matmul(
                                psy[:],
                                mt[:, kd, mc * P:(mc + 1) * P],
                                vsb[:, kd, c * 512:(c + 1) * 512],
                                start=(kd == 0), stop=(kd == 2))
                        ych = ych_pool.tile([P, 512], dt.float32, tag="ych",
                                            name="ych")
                        nc.scalar.activation(ych[:], psy[:], Act.Identity,
                                             bias=bp_sb[:, mc:mc + 1])
                        nc.sync.dma_start(
                            yout[mc * P:(mc + 1) * P,
                                 MEGA * m + c * 512:MEGA * m + (c + 1) * 512],
                            ych[:])


_NC_CACHE = {}


def _get_nc():
    if "nc" not in _NC_CACHE:
        _NC_CACHE["nc"] = build_nc()
    return _NC_CACHE["nc"]


def kernel(**inputs):
    x = np.asarray(inputs["x"], dtype=np.float32)            # (8, 384, 128, 128)
    qkv_w = np.asarray(inputs["qkv_w"], dtype=np.float32).reshape(1152, 384)
    qkv_b = np.asarray(inputs["qkv_b"], dtype=np.float32)
    dw_w = np.asarray(inputs["dw_w"], dtype=np.float32).reshape(1152, 9)
    dw_b = np.asarray(inputs["dw_b"], dtype=np.float32)
    proj_w = np.asarray(inputs["proj_w"], dtype=np.float32).reshape(384, 384)
    proj_b = np.asarray(inputs["proj_b"], dtype=np.float32)
    temperature = np.asarray(inputs["temperature"], dtype=np.float32).reshape(8)

    nc = _get_nc()
    B = x.shape[0]
    shared = {
        "wqkv": qkv_w, "bqkv": qkv_b, "wdw": dw_w, "bdw": dw_b,
        "wp": proj_w, "bp": proj_b, "temp": temperature,
    }
    in_maps = [dict(shared, x=np.ascontiguousarray(x[b].reshape(CI, HW)))
               for b in range(B)]
    res = run_bass_kernel_spmd(nc, in_maps, list(range(NCORES)), trace=False)
    out = np.stack([res.results[b]["y"].reshape(CI, 128, 128) for b in range(B)])
    return out


# revision 11
# speedup vs baseline: 1.0696x; 1.0696x over previous
"""Trainium2 Bass kernel for channel attention (1x1 conv -> depthwise 3x3 ->
per-head channel attention over pixels -> 1x1 projection).

Data-parallel over batch: 8 images -> 8 NeuronCores, no collectives.
Self-contained: hardcodes shapes from the problem spec.
"""
import sys

sys.path.insert(0, "/opt/trn_rl_repo")

import numpy as np  # noqa: E402

import concourse.bacc as bacc  # noqa: E402
import concourse.mybir as mybir  # noqa: E402
from concourse import masks  # noqa: E402
from concourse.tile import TileContext  # noqa: E402
from concourse.bass_utils import run_bass_kernel_spmd  # noqa: E402

dt = mybir.dt
Alu = mybir.AluOpType
Act = mybir.ActivationFunctionType
Axis = mybir.AxisListType
F32R = dt.float32r

# geometry
P = 128
W = 128            # image row length
HW = 16384         # pixels per image
MEGA = 2048        # pixels per mega-tile (16 image rows)
ROWS = MEGA // W   # 16
NMEGA = HW // MEGA  # 8
ZW = MEGA + 2 * W  # 2304 data cols: mega + 1 halo row each side
ZT = 2432          # z tile width: 1 left pad + ZW data + right pad (19 rows)
ZB = 129           # z col of first output pixel (out c -> z col c + ZB)
CI = 384
HEADS = 8
CH = 48            # channels per head
NCORES = 8

# dwconv tap split: (dy, dx); flat shift = 128*dy + dx on the padded z layout
PE_TAPS_QK = [(0, -1), (0, 1), (-1, -1), (-1, 1)]
DVE_TAPS_QK = [(1, 0), (0, 0), (-1, 0), (1, -1), (1, 1)]  # first: dx == 0
PE_TAPS_V = [(0, 0), (0, -1), (0, 1), (-1, 0), (-1, -1)]
DVE_TAPS_V = [(1, 0), (1, -1), (1, 1), (-1, 1)]  # first: dx == 0

# banded gram layout: for q-chunk g (128 q channels), k-channel band
BANDS = [(0, 144), (96, 192), (240, 144)]  # (start, width) in k channels
# per-head partition segments: (gchunk, p0, p1, col0, head)
SEGMENTS = [
    (0, 0, 48, 0, 0), (0, 48, 96, 48, 1), (0, 96, 128, 96, 2),
    (1, 0, 16, 0, 2), (1, 16, 64, 48, 3), (1, 64, 112, 96, 4),
    (1, 112, 128, 144, 5),
    (2, 0, 32, 0, 5), (2, 32, 80, 48, 6), (2, 80, 128, 96, 7),
]


def _mega_geometry(m):
    """(w0, conv_chunks) for mega m. w0: DRAM px of window col 0 (may be <0).
    conv_chunks: (off, n) over valid window cols [z_lo, z_hi)."""
    w0 = MEGA * m - W
    z_lo = W if m == 0 else 0
    z_hi = ZW - W if m == NMEGA - 1 else ZW
    chunks = []
    off = z_lo
    while off < z_hi:
        n = min(512, z_hi - off)
        chunks.append((off, n))
        off += n
    return w0, chunks


def build_nc():
    nc = bacc.Bacc("TRN2", target_bir_lowering=False, debug=False)

    xin = nc.dram_tensor("x", [CI, HW], F32R, kind="ExternalInput").ap()
    wqkv = nc.dram_tensor("wqkv", [3 * CI, CI], dt.float32, kind="ExternalInput").ap()
    bqkv = nc.dram_tensor("bqkv", [3 * CI], dt.float32, kind="ExternalInput").ap()
    wdw = nc.dram_tensor("wdw", [3 * CI, 9], dt.float32, kind="ExternalInput").ap()
    bdw = nc.dram_tensor("bdw", [3 * CI], dt.float32, kind="ExternalInput").ap()
    wp = nc.dram_tensor("wp", [CI, CI], dt.float32, kind="ExternalInput").ap()
    bp = nc.dram_tensor("bp", [CI], dt.float32, kind="ExternalInput").ap()
    temp = nc.dram_tensor("temp", [HEADS], dt.float32, kind="ExternalInput").ap()
    yout = nc.dram_tensor("y", [CI, HW], dt.float32, kind="ExternalOutput").ap()

    with TileContext(nc) as tc:
        _build(tc, nc, xin, wqkv, bqkv, wdw, bdw, wp, bp, temp, yout)
    nc.compile()
    return nc


def _conv1x1(nc, ps_pool, wqkvT, xw, z, g, bias, chunks):
    """1x1 conv for output-channel chunk g into padded z tile."""
    for off, n in chunks:
        psc = ps_pool.tile([P, 512], dt.float32, tag="psc", name="psc")
        for c3 in range(3):
            nc.tensor.matmul(
                psc[:, :n],
                wqkvT[:, c3, g * P:(g + 1) * P],
                xw[:, c3, off:off + n],
                start=(c3 == 0), stop=(c3 == 2))
        nc.scalar.activation(z[:, off + 1:off + 1 + n], psc[:, :n],
                             Act.Identity, bias=bias)


def _dwconv(nc, ps_dw, acc_pool, z, g, dww, dww_neg, diags, ident_mm,
            bias, evac_out, pe_taps, dve_taps, bf16):
    """Depthwise 3x3 on padded z -> 4 psum chunks, evacuated via
    evac_out(c). Taps split across PE (diag matmuls) and DVE (flat-shift
    scalar_tensor_tensor into an sbuf accumulator, merged by an identity
    matmul). Row-wrap artifacts of the flat shifts are corrected on acc."""
    acc_dt = dt.bfloat16 if bf16 else F32R
    zf = z if bf16 else z.bitcast(dt.float32)
    z2 = zf.rearrange("p (r x) -> p r x", x=W)  # 19 rows
    acc = acc_pool.tile([P, MEGA], acc_dt, tag="acc", name="acc")
    accf = acc if bf16 else acc.bitcast(dt.float32)
    for i, (dy, dx) in enumerate(dve_taps):
        t = (dy + 1) * 3 + (dx + 1)
        wsc = dww[:, g, t:t + 1]
        src = zf[:, ZB + 128 * dy + dx: ZB + 128 * dy + dx + MEGA]
        if i == 0:
            assert dx == 0
            nc.vector.tensor_scalar_mul(acc[:], src, wsc)
        else:
            nc.vector.scalar_tensor_tensor(acc[:], src, wsc, accf[:],
                                           Alu.mult, Alu.add)
    # wrap corrections for every dx != 0 tap (PE taps included: acc is
    # merged into the psum, so linear corrections can all land on acc)
    acc3o = acc.rearrange("p (r x) -> p r x", x=W)   # native-dtype out view
    acc3f = accf.rearrange("p (r x) -> p r x", x=W)  # readable view
    for (dy, dx) in pe_taps + dve_taps:
        if dx == 0:
            continue
        t = (dy + 1) * 3 + (dx + 1)
        wneg = dww_neg[:, g, t:t + 1]
        if dx == -1:
            o_ap, i_ap = acc3o[:, :, 0:1], acc3f[:, :, 0:1]
            s_ap = z2[:, dy + 1:dy + 17, 0:1]
        else:
            o_ap, i_ap = acc3o[:, :, W - 1:W], acc3f[:, :, W - 1:W]
            s_ap = z2[:, dy + 2:dy + 18, 1:2]
        nc.vector.scalar_tensor_tensor(o_ap, s_ap, wneg, i_ap,
                                       Alu.mult, Alu.add)
    for c in range(4):
        psd = ps_dw.tile([P, 512], dt.float32, tag="psd", name="psd")
        for i, (dy, dx) in enumerate(pe_taps):
            s0 = ZB + 512 * c + 128 * dy + dx
            nc.tensor.matmul(psd[:], diags[:, g % 6, i, :],
                             z[:, s0:s0 + 512],
                             start=(i == 0), stop=False)
        nc.tensor.matmul(psd[:], ident_mm[:],
                         acc[:, 512 * c:512 * (c + 1)],
                         start=False, stop=True)
        nc.scalar.activation(evac_out(c), psd[:], Act.Identity, bias=bias)


def _build(tc, nc, xin, wqkv, bqkv, wdw, bdw, wp, bp, temp, yout):
    from contextlib import ExitStack

    es = ExitStack()
    with es:
        const = es.enter_context(tc.tile_pool(name="const", bufs=1))
        wsetup = es.enter_context(tc.tile_pool(name="wsetup", bufs=2))
        xw_pool = es.enter_context(tc.tile_pool(name="xw", bufs=2))
        z_pool = es.enter_context(tc.tile_pool(name="z", bufs=2))
        acc_pool = es.enter_context(tc.tile_pool(name="acc", bufs=2))
        small = es.enter_context(tc.tile_pool(name="small", bufs=2))
        ps_conv = es.enter_context(tc.tile_pool(name="ps_conv", bufs=2, space="PSUM"))
        ps_dw = es.enter_context(tc.tile_pool(name="ps_dw", bufs=2, space="PSUM"))

        # ---------------- constants / weights ----------------
        ident_f = const.tile([P, P], dt.float32)
        masks.make_identity(nc, ident_f[:])
        ident_bf = const.tile([P, P], dt.bfloat16)
        masks.make_identity(nc, ident_bf[:])
        ident_r = const.tile([P, P], F32R)
        nc.scalar.activation(ident_r[:], ident_f[:], Act.Copy)

        wqkvT = const.tile([P, 3, 3 * CI], F32R)   # [ci_in, ci_chunk, co]
        for o9 in range(9):
            wtmp = wsetup.tile([P, CI], dt.float32, tag="wtmp", name="wtmp")
            nc.sync.dma_start(wtmp[:], wqkv[o9 * P:(o9 + 1) * P, :])
            for c3 in range(3):
                pst = ps_conv.tile([P, 512], dt.float32, tag="psc", name="pst")
                nc.tensor.transpose(pst[:, :P], wtmp[:, c3 * P:(c3 + 1) * P],
                                    ident_f[:])
                nc.scalar.activation(wqkvT[:, c3, o9 * P:(o9 + 1) * P],
                                     pst[:, :P], Act.Copy)

        wpT = const.tile([P, 3, CI], F32R)  # [e, e_chunk, c]
        for mc in range(3):
            wtmp = wsetup.tile([P, CI], dt.float32, tag="wtmp", name="wtmp")
            nc.sync.dma_start(wtmp[:], wp[mc * P:(mc + 1) * P, :])
            for ec in range(3):
                pst = ps_conv.tile([P, 512], dt.float32, tag="psc", name="pst")
                nc.tensor.transpose(pst[:, :P], wtmp[:, ec * P:(ec + 1) * P],
                                    ident_f[:])
                nc.scalar.activation(wpT[:, ec, mc * P:(mc + 1) * P],
                                     pst[:, :P], Act.Copy)

        dww = const.tile([P, 9, 9], dt.float32)  # [ch, g, tap]
        for g in range(9):
            nc.sync.dma_start(dww[:, g, :], wdw[g * P:(g + 1) * P, :])
        dww_neg = const.tile([P, 9, 9], dt.float32)
        nc.vector.tensor_scalar_mul(dww_neg[:], dww[:], -1.0)

        bqkv_sb = const.tile([P, 9], dt.float32)
        nc.sync.dma_start(bqkv_sb[:], bqkv.rearrange("(g p) -> p g", p=P))
        bdw_sb = const.tile([P, 9], dt.float32)
        nc.sync.dma_start(bdw_sb[:], bdw.rearrange("(g p) -> p g", p=P))
        bp_sb = const.tile([P, 3], dt.float32)
        nc.sync.dma_start(bp_sb[:], bp.rearrange("(g p) -> p g", p=P))

        # per-q-channel temperature via broadcast segment DMAs
        tmap = const.tile([P, 3], dt.float32)
        for (gq, p0, p1, c0, h) in SEGMENTS:
            src = temp[h:h + 1].rearrange("(a b) -> a b", b=1)
            nc.sync.dma_start(tmap[p0:p1, gq:gq + 1],
                              src.to_broadcast([p1 - p0, 1]))

        # diag tap-weight matrices for PE dwconv taps
        diags_bf = const.tile([P, 6, len(PE_TAPS_QK), P], dt.bfloat16)
        for g in range(6):
            for i, (dy, dx) in enumerate(PE_TAPS_QK):
                t = (dy + 1) * 3 + (dx + 1)
                nc.vector.tensor_scalar_mul(diags_bf[:, g, i, :], ident_f[:],
                                            dww[:, g, t:t + 1])
        diags_r = const.tile([P, 3, len(PE_TAPS_V), P], F32R)
        for g in range(3):
            for i, (dy, dx) in enumerate(PE_TAPS_V):
                t = (dy + 1) * 3 + (dx + 1)
                nc.vector.tensor_scalar_mul(diags_r[:, g, i, :], ident_f[:],
                                            dww[:, 6 + g, t:t + 1])

        ones_col = const.tile([P, 1], dt.float32)
        nc.gpsimd.memset(ones_col[:], 1.0)
        ones_row = const.tile([1, P], dt.float32)
        nc.gpsimd.memset(ones_row[:], 1.0)

        sqp = const.tile([P, 6, NMEGA], dt.float32)
        a_bd = const.tile([P, 3, CI], F32R)
        mt = const.tile([P, 3, CI], F32R)
        logit_all = const.tile([P, 3, 192], dt.float32)

        # ---------------- phase 1: q/k ----------------
        with ExitStack() as qk_es:
            gram_pool = qk_es.enter_context(
                tc.tile_pool(name="gram", bufs=1, space="PSUM"))
            ps_t = qk_es.enter_context(
                tc.tile_pool(name="ps_t", bufs=1, space="PSUM"))
            qbf_pool = qk_es.enter_context(tc.tile_pool(name="qbf", bufs=3))
            qkt_pool = qk_es.enter_context(tc.tile_pool(name="qkt", bufs=1))
            scr_pool = qk_es.enter_context(tc.tile_pool(name="scr", bufs=2))

            grams = [gram_pool.tile([P, BANDS[g][1]], dt.float32, tag=f"g{g}",
                                    name=f"gram{g}")
                     for g in range(3)]

            for m in range(NMEGA):
                w0, chunks = _mega_geometry(m)
                xw = xw_pool.tile([P, 3, ZW], F32R, tag="xw", name="xw")
                xs = max(w0, 0)
                xe = min(w0 + ZW, HW)
                for c3 in range(3):
                    nc.sync.dma_start(xw[:, c3, xs - w0:xe - w0],
                                      xin[c3 * P:(c3 + 1) * P, xs:xe])

                qkt = qkt_pool.tile([P, ROWS, 768], dt.bfloat16, tag="qkt",
                                    name="qkt")

                for g in range(6):
                    z = z_pool.tile([P, ZT], dt.bfloat16, tag="zq", name="z")
                    nc.gpsimd.memset(z[:, 0:1], 0.0)
                    nc.gpsimd.memset(z[:, 1 + ZW:ZT], 0.0)
                    if m == 0:
                        nc.gpsimd.memset(z[:, 1:1 + W], 0.0)
                    if m == NMEGA - 1:
                        nc.gpsimd.memset(z[:, 1 + ZW - W:1 + ZW], 0.0)
                    _conv1x1(nc, ps_conv, wqkvT, xw, z, g,
                             bqkv_sb[:, g:g + 1], chunks)

                    qb = qbf_pool.tile([P, MEGA], dt.bfloat16, tag="qb",
                                       name="qb")
                    _dwconv(nc, ps_dw, acc_pool, z, g, dww, dww_neg, diags_bf,
                            ident_bf, bdw_sb[:, g:g + 1],
                            lambda c: qb[:, 512 * c:512 * (c + 1)],
                            PE_TAPS_QK, DVE_TAPS_QK, True)

                    # sum of squares for the l2 norms
                    scr = scr_pool.tile([P, MEGA], dt.bfloat16, tag="scr",
                                        name="scr")
                    nc.scalar.activation(scr[:], qb[:], Act.Square,
                                         accum_out=sqp[:, g, m:m + 1])

                    # transpose to [px, ch] via PE, evacuate on DVE
                    for c in range(4):
                        pstr = ps_t.tile([P, 512], dt.bfloat16, tag="pstr",
                                         name="pstr")
                        for j in range(4):
                            blk = 4 * c + j
                            nc.tensor.transpose(pstr[:, j * P:(j + 1) * P],
                                                qb[:, blk * P:(blk + 1) * P],
                                                ident_bf[:])
                        nc.vector.tensor_copy(
                            qkt[:, 4 * c:4 * c + 4, g * P:(g + 1) * P],
                            pstr[:])

                # banded gram accumulation over px chunks
                for j in range(ROWS):
                    for gq in range(3):
                        bs, bw = BANDS[gq]
                        nc.tensor.matmul(
                            grams[gq][:],
                            qkt[:, j, gq * P:(gq + 1) * P],
                            qkt[:, j, CI + bs:CI + bs + bw],
                            start=(m == 0 and j == 0),
                            stop=(m == NMEGA - 1 and j == ROWS - 1))

            # ------------- phase 2a: norms, logits, softmax, A_bd -------------
            sq = const.tile([P, 6], dt.float32)
            nc.vector.tensor_reduce(sq[:], sqp[:], Axis.X, Alu.add)
            nrm = const.tile([P, 6], dt.float32)
            nc.scalar.activation(nrm[:], sq[:], Act.Sqrt)
            inv = const.tile([P, 6], dt.float32)
            nc.vector.reciprocal(inv[:], nrm[:])
            rowscale = const.tile([P, 3], dt.float32)
            nc.vector.tensor_tensor(rowscale[:], tmap[:], inv[:, 0:3], Alu.mult)

            # broadcast 1/||k|| along partitions: invk_bc[p, d] = inv_nk[d]
            invk_row = const.tile([1, CI], dt.float32)
            for g in range(3):
                dtmp = small.tile([P, P], dt.float32, tag="dtmp", name="dtmp")
                nc.vector.tensor_scalar_mul(dtmp[:], ident_f[:],
                                            inv[:, 3 + g:4 + g])
                pst = ps_conv.tile([P, 512], dt.float32, tag="psc", name="pst")
                nc.tensor.matmul(pst[:1, :P], ones_col[:], dtmp[:],
                                 start=True, stop=True)
                nc.scalar.activation(invk_row[:, g * P:(g + 1) * P],
                                     pst[:1, :P], Act.Copy)
            invk_bc = const.tile([P, CI], dt.float32)
            pst2 = ps_dw.tile([P, 512], dt.float32, tag="psd", name="pst2")
            nc.tensor.matmul(pst2[:, :CI], ones_row[:], invk_row[:],
                             start=True, stop=True)
            nc.scalar.activation(invk_bc[:], pst2[:, :CI], Act.Copy)

            for gq in range(3):
                bs, bw = BANDS[gq]
                nc.scalar.activation(logit_all[:, gq, :bw], grams[gq][:],
                                     Act.Identity,
                                     scale=rowscale[:, gq:gq + 1])
                nc.vector.tensor_tensor(logit_all[:, gq, :bw],
                                        logit_all[:, gq, :bw],
                                        invk_bc[:, bs:bs + bw], Alu.mult)

            # head-aligned softmax: gather [48 qc, 8 heads, 48 kc]
            L = const.tile([CH, HEADS, CH], dt.float32)
            for (gq, p0, p1, c0, h) in SEGMENTS:
                lp0 = 128 * gq + p0 - CH * h
                nc.sync.dma_start(L[lp0:lp0 + (p1 - p0), h, :],
                                  logit_all[p0:p1, gq, c0:c0 + CH])
            nmx = const.tile([CH, HEADS], dt.float32)
            nc.vector.tensor_reduce(nmx[:], L[:], Axis.X, Alu.max, negate=True)
            nc.vector.tensor_tensor(L[:], L[:],
                                    nmx[:, :, None].to_broadcast(
                                        (CH, HEADS, CH)), Alu.add)
            Lx = const.tile([CH, HEADS, CH], dt.float32)
            nc.scalar.activation(Lx[:], L[:], Act.Exp)
            ssum = const.tile([CH, HEADS], dt.float32)
            nc.vector.tensor_reduce(ssum[:], Lx[:], Axis.X, Alu.add)
            rinv = const.tile([CH, HEADS], dt.float32)
            nc.vector.reciprocal(rinv[:], ssum[:])
            Lxr = const.tile([CH, HEADS, CH], F32R)
            nc.vector.tensor_tensor(Lxr[:], Lx[:],
                                    rinv[:, :, None].to_broadcast(
                                        (CH, HEADS, CH)), Alu.mult)

            for gz in range(3):
                nc.vector.tensor_scalar_mul(
                    a_bd[:, gz, :], wpT[:, 0, :].bitcast(dt.float32), 0.0)
            for (gq, p0, p1, c0, h) in SEGMENTS:
                lp0 = 128 * gq + p0 - CH * h
                nc.sync.dma_start(a_bd[p0:p1, gq, CH * h:CH * (h + 1)],
                                  Lxr[lp0:lp0 + (p1 - p0), h, :])

        # ---------------- phase 2b: MT = (Wp @ A_bd)^T ----------------
        for dc in range(3):
            psm = ps_conv.tile([P, 512], dt.float32, tag="psc", name="psm")
            for ec in range(3):
                nc.tensor.matmul(psm[:, :CI],
                                 a_bd[:, ec, dc * P:(dc + 1) * P],
                                 wpT[:, ec, :],
                                 start=(ec == 0), stop=(ec == 2))
            nc.scalar.activation(mt[:, dc, :], psm[:, :CI], Act.Copy)

        # ---------------- phase 3: v and output ----------------
        with ExitStack() as v_es:
            vsb_pool = v_es.enter_context(tc.tile_pool(name="vsb", bufs=2))
            ych_pool = v_es.enter_context(tc.tile_pool(name="ych", bufs=2))
            ps_mv = v_es.enter_context(
                tc.tile_pool(name="ps_mv", bufs=2, space="PSUM"))

            for m in range(NMEGA):
                w0, chunks = _mega_geometry(m)
                xw = xw_pool.tile([P, 3, ZW], F32R, tag="xw", name="xw")
                xs = max(w0, 0)
                xe = min(w0 + ZW, HW)
                for c3 in range(3):
                    nc.sync.dma_start(xw[:, c3, xs - w0:xe - w0],
                                      xin[c3 * P:(c3 + 1) * P, xs:xe])

                vsb = vsb_pool.tile([P, 3, MEGA], F32R, tag="vsb",
                                    name="vsb")

                for gv in range(3):
                    g = 6 + gv
                    z = z_pool.tile([P, ZT], F32R, tag="z", name="z")
                    nc.vector.tensor_scalar_mul(z[:, 0:1], ident_f[:, 0:1], 0.0)
                    nc.vector.tensor_scalar_mul(z[:, 1 + ZW:ZT],
                                                ident_f[:, 0:ZT - ZW - 1], 0.0)
                    if m == 0:
                        nc.vector.tensor_scalar_mul(z[:, 1:1 + W], ident_f[:],
                                                    0.0)
                    if m == NMEGA - 1:
                        nc.vector.tensor_scalar_mul(z[:, 1 + ZW - W:1 + ZW],
                                                    ident_f[:], 0.0)
                    _conv1x1(nc, ps_conv, wqkvT, xw, z, g,
                             bqkv_sb[:, g:g + 1], chunks)

                    _dwconv(nc, ps_dw, acc_pool, z, 6 + gv, dww, dww_neg,
                            diags_r, ident_r, bdw_sb[:, g:g + 1],
                            lambda c, gv=gv: vsb[:, gv, 512 * c:512 * (c + 1)],
                            PE_TAPS_V, DVE_TAPS_V, False)

                # y = M @ v + bias
                for mc in range(3):
                    for c in range(4):
                        psy = ps_mv.tile([P, 512], dt.float32, tag="psy",
                                         name="psy")
                        for kd in range(3):
                            nc.tensor.matmul(
                                psy[:],
                                mt[:, kd, mc * P:(mc + 1) * P],
                                vsb[:, kd, c * 512:(c + 1) * 512],
                                start=(kd == 0), stop=(kd == 2))
                        ych = ych_pool.tile([P, 512], dt.float32, tag="ych",
                                            name="ych")
                        nc.scalar.activation(ych[:], psy[:], Act.Identity,
                                             bias=bp_sb[:, mc:mc + 1])
                        nc.sync.dma_start(
                            yout[mc * P:(mc + 1) * P,
                                 MEGA * m + c * 512:MEGA * m + (c + 1) * 512],
                            ych[:])


_NC_CACHE = {}


def _get_nc():
    if "nc" not in _NC_CACHE:
        _NC_CACHE["nc"] = build_nc()
    return _NC_CACHE["nc"]


def kernel(**inputs):
    x = np.asarray(inputs["x"], dtype=np.float32)            # (8, 384, 128, 128)
    qkv_w = np.asarray(inputs["qkv_w"], dtype=np.float32).reshape(1152, 384)
    qkv_b = np.asarray(inputs["qkv_b"], dtype=np.float32)
    dw_w = np.asarray(inputs["dw_w"], dtype=np.float32).reshape(1152, 9)
    dw_b = np.asarray(inputs["dw_b"], dtype=np.float32)
    proj_w = np.asarray(inputs["proj_w"], dtype=np.float32).reshape(384, 384)
    proj_b = np.asarray(inputs["proj_b"], dtype=np.float32)
    temperature = np.asarray(inputs["temperature"], dtype=np.float32).reshape(8)

    nc = _get_nc()
    B = x.shape[0]
    shared = {
        "wqkv": qkv_w, "bqkv": qkv_b, "wdw": dw_w, "bdw": dw_b,
        "wp": proj_w, "bp": proj_b, "temp": temperature,
    }
    in_maps = [dict(shared, x=np.ascontiguousarray(x[b].reshape(CI, HW)))
               for b in range(B)]
    res = run_bass_kernel_spmd(nc, in_maps, list(range(NCORES)), trace=False)
    out = np.stack([res.results[b]["y"].reshape(CI, 128, 128) for b in range(B)])
    return out
